# revision 11
# baseline (speedup 1.0000x reference)
"""VQ codebook bottleneck block on 8 Trainium2 NeuronCores.

Full inputs in, full outputs out. Data-parallel over the batch axis:
core c handles x[c] (8192 tokens of dim 64); the [2048, 64] codebook is
replicated. The device computes, per core, the argmin codes (via a
single fused matmul S = 2*x@k.T - ||k||^2 and a max/max-index search)
and gathers xq = k[codes]. The tiny EMA/scatter update is combined on
the host.

Self-contained: shapes/sharding hardcoded for
x:[8,64,8192] k:[2048,64] k_sum:[2048,64] k_elem:[2048].
"""

import numpy as np

import concourse.bass as bass
import concourse.bacc as bacc
import concourse.mybir as mybir
from concourse.tile import TileContext
from concourse import bass_utils

MU = 0.99
THRESHOLD = 1.0

N, D, T, K = 8, 64, 8192, 2048
M = T              # tokens per core (one batch entry per core)
MT = M // 128      # m-tiles per core
NCORES = 8
NG = 8             # gather groups
SG = MT // NG      # m-tiles per group

F32 = mybir.dt.float32
F32R = mybir.dt.float32r
BF16 = mybir.dt.bfloat16
FP16 = mybir.dt.float16
U16 = mybir.dt.uint16
I16 = mybir.dt.int16

# matmul mode: "bf16x2" | "fp16x2" (3-term hi/lo splits, 1 cyc/row each
# term), "fp32" (exact, 4 cyc/row), "fp32r" (tf32-class, too lossy).
MATMUL_MODE = "bf16x2"
# m-tiles with s % GPSIMD_MAX_PERIOD == 0 compute the row max via DVE
# max8; others use a GPSIMD tensor_tensor max tree (load balancing).
# 0 => all tiles use DVE max8.
GPSIMD_MAX_PERIOD = 0


def _hi_lo(a, dt):
    np_dt = np.dtype(mybir.dt.np(dt))
    if np_dt == np.float16:
        hi = np.asarray(a, np.float32).astype(np.float16)
        lo = (np.asarray(a, np.float32) - hi.astype(np.float32)).astype(np.float16)
        return hi, lo
    # bf16 via bit truncation with round-to-nearest
    a32 = np.asarray(a, np.float32)
    b = a32.view(np.uint32)
    hi = ((b + 0x8000) & 0xFFFF0000).view(np.float32)
    lo = a32 - hi
    return hi.astype(np_dt), lo.astype(np_dt)


def build_nc(mode=MATMUL_MODE, gpsimd_period=GPSIMD_MAX_PERIOD):
    nc = bacc.Bacc("TRN2", target_bir_lowering=False, debug=False)
    lowp = mode in ("bf16x2", "fp16x2")
    HDT = BF16 if mode == "bf16x2" else FP16

    # ---- I/O ----
    if lowp:
        xplus_hi = nc.dram_tensor("xplus_hi", [D + 1, M], HDT, kind="ExternalInput")
        xplus_lo = nc.dram_tensor("xplus_lo", [D + 1, M], HDT, kind="ExternalInput")
        w_hi = nc.dram_tensor("w_hi", [D + 1, K], HDT, kind="ExternalInput")
        w_lo = nc.dram_tensor("w_lo", [D + 1, K], HDT, kind="ExternalInput")
    else:
        _xt = F32R if mode == "fp32r" else F32
        xplus = nc.dram_tensor("xplus", [D + 1, M], _xt, kind="ExternalInput")
        w = nc.dram_tensor("w", [D + 1, K], _xt, kind="ExternalInput")
    kg = nc.dram_tensor("kg", [K, D], F32, kind="ExternalInput")

    xq_out = nc.dram_tensor("xq_out", [M, D], F32, kind="ExternalOutput")
    codes_out = nc.dram_tensor("codes_out", [M], U16, kind="ExternalOutput")

    with TileContext(nc) as tc:
        with (
            tc.tile_pool(name="persist", bufs=1) as persist,
            tc.tile_pool(name="spsum", bufs=2, space="PSUM") as spsum,
            tc.tile_pool(name="ssb", bufs=3) as ssb,
            tc.tile_pool(name="small", bufs=4) as small,
        ):
            # ---- persistent SBUF ----
            if lowp:
                xp_hi_sb = persist.tile([D + 1, M], HDT, tag="xp_hi")
                xp_lo_sb = persist.tile([D + 1, M], HDT, tag="xp_lo")
                w_hi_sb = persist.tile([D + 1, K], HDT, tag="w_hi")
                w_lo_sb = persist.tile([D + 1, K], HDT, tag="w_lo")
                nc.sync.dma_start(xp_hi_sb[:], xplus_hi[:])
                nc.sync.dma_start(xp_lo_sb[:], xplus_lo[:])
                nc.sync.dma_start(w_hi_sb[:], w_hi[:])
                nc.sync.dma_start(w_lo_sb[:], w_lo[:])
            else:
                _xt = F32R if mode == "fp32r" else F32
                xp_sb = persist.tile([D + 1, M], _xt, tag="xp")
                w_sb = persist.tile([D + 1, K], _xt, tag="w")
                nc.sync.dma_start(xp_sb[:], xplus[:])
                nc.sync.dma_start(w_sb[:], w[:])

            codes8_sb = persist.tile([128, MT, 8], U16, tag="codes8")
            idxs_sb = persist.tile([128, M // 16], U16, tag="idxs")
            xq_sb = persist.tile([128, MT, D], F32, tag="xq")
            zeros8 = persist.tile([128, 8], F32, tag="zeros8")
            nc.gpsimd.memset(zeros8[:], 0.0)

            # ---- main loop over m-tiles, gather pipelined per group ----
            for g in range(NG):
                for s in range(g * SG, (g + 1) * SG):
                    s_psum = spsum.tile([128, K], F32, tag="S")
                    s_sb = ssb.tile([128, K], F32, tag="Ssb")
                    in_max8 = small.tile([128, 8], F32, tag="max8")

                    for j in range(K // 512):
                        js = slice(j * 512, (j + 1) * 512)
                        xsl = slice(s * 128, (s + 1) * 128)
                        if lowp:
                            nc.tensor.matmul(
                                s_psum[:, js], xp_hi_sb[:, xsl], w_hi_sb[:, js],
                                start=True, stop=False,
                            )
                            nc.tensor.matmul(
                                s_psum[:, js], xp_hi_sb[:, xsl], w_lo_sb[:, js],
                                start=False, stop=False,
                            )
                            nc.tensor.matmul(
                                s_psum[:, js], xp_lo_sb[:, xsl], w_hi_sb[:, js],
                                start=False, stop=True,
                            )
                        else:
                            nc.tensor.matmul(
                                s_psum[:, js], xp_sb[:, xsl], w_sb[:, js],
                                start=True, stop=True,
                            )

                    nc.scalar.copy(s_sb[:], s_psum[:])

                    use_gpsimd = gpsimd_period and (s % gpsimd_period != 0)
                    if use_gpsimd:
                        tprev = s_sb
                        width = K
                        while width > 1:
                            width //= 2
                            tnew = small.tile([128, max(width, 8)], F32,
                                              tag=f"tree{width}")
                            nc.gpsimd.tensor_tensor(
                                tnew[:, :width],
                                tprev[:, :width],
                                tprev[:, width:2 * width],
                                op=mybir.AluOpType.max,
                            )
                            tprev = tnew
                        nc.gpsimd.tensor_scalar(
                            in_max8[:], zeros8[:], tprev[:, 0:1], None,
                            op0=mybir.AluOpType.add,
                        )
                    else:
                        nc.vector.max(in_max8[:], s_sb[:])

                    nc.vector.max_index(codes8_sb[:, s, :], in_max8[:], s_sb[:])

                # -- group tail: shuffle codes into idx layout, gather --
                gsl = slice(g * SG, (g + 1) * SG)
                isl = slice(g * (M // NG // 16), (g + 1) * (M // NG // 16))
                csl = slice(g * (M // NG), (g + 1) * (M // NG))
                # bounce through DRAM to cross partitions (also the output)
                nc.sync.dma_start(
                    codes_out[csl].rearrange("(s p) -> p s", p=128),
                    codes8_sb[:, gsl, 0],
                )
                for r in range(8):
                    nc.sync.dma_start(
                        idxs_sb[r * 16:(r + 1) * 16, isl],
                        codes_out[csl].rearrange("(c q) -> q c", q=16),
                    )
                nc.gpsimd.dma_gather(
                    xq_sb[:, gsl, :], kg[:], idxs_sb[:, isl].bitcast(I16),
                    M // NG, M // NG, D, queue_num=0,
                )
                nc.sync.dma_start(
                    xq_out[:].rearrange("(s p) d -> p s d", p=128)[:, gsl, :],
                    xq_sb[:, gsl, :],
                )

    nc.compile()
    return nc


_NC_CACHE = {}


def _get_nc(mode=MATMUL_MODE, gpsimd_period=GPSIMD_MAX_PERIOD):
    key = (mode, gpsimd_period)
    if key not in _NC_CACHE:
        _NC_CACHE[key] = build_nc(mode, gpsimd_period)
    return _NC_CACHE[key]


def make_in_maps(x, k, mode=MATMUL_MODE):
    x = np.asarray(x, np.float32)
    k = np.asarray(k, np.float32)
    kk = (k.astype(np.float64) ** 2).sum(1).astype(np.float32)
    w_full = np.concatenate([2.0 * k.T, -kk[None, :]], axis=0)  # [65, K]
    ones = np.ones((1, M), np.float32)
    lowp = mode in ("bf16x2", "fp16x2")
    HDT = BF16 if mode == "bf16x2" else FP16
    if lowp:
        wh, wl = _hi_lo(w_full, HDT)
    kc = np.ascontiguousarray(k)
    in_maps = []
    for c in range(NCORES):
        xplus = np.concatenate([x[c], ones], axis=0)  # [65, M]
        m = {"kg": kc}
        if lowp:
            xh, xl = _hi_lo(xplus, HDT)
            m["xplus_hi"] = xh
            m["xplus_lo"] = xl
            m["w_hi"] = wh
            m["w_lo"] = wl
        else:
            m["xplus"] = np.ascontiguousarray(xplus)
            m["w"] = np.ascontiguousarray(w_full)
        in_maps.append(m)
    return in_maps


def postprocess(results, x, k, k_sum, k_elem):
    """results: list of 8 dicts with xq_out, codes_out."""
    x = np.asarray(x, np.float32)
    k = np.asarray(k, np.float32)
    k_sum = np.asarray(k_sum, np.float32)
    k_elem = np.asarray(k_elem, np.float32)

    xq = np.stack([results[c]["xq_out"] for c in range(NCORES)])  # [8, T, D]
    codes = np.stack(
        [results[c]["codes_out"].astype(np.int32) for c in range(NCORES)]
    )  # [8, T]

    xd = np.ascontiguousarray(xq.transpose(0, 2, 1)).astype(np.float32)

    # commitment loss
    xf = x.transpose(0, 2, 1).reshape(-1, D)
    xqf = xq.reshape(-1, D)
    diff = xqf.astype(np.float64) - xf.astype(np.float64)
    commit = np.float32(np.mean(diff * diff))

    # segment sums on host (device scatter-add RMW races on dup indices)
    flat_codes = codes.reshape(-1).astype(np.int64)
    cnt_new = np.bincount(flat_codes, minlength=K).astype(np.float32)
    sum_new = np.empty((K, D), np.float32)
    xf64 = xf.astype(np.float64)
    for d in range(D):
        sum_new[:, d] = np.bincount(flat_codes, weights=xf64[:, d], minlength=K)

    k_sum_n = (MU * k_sum + (1.0 - MU) * sum_new).astype(np.float32)
    k_elem_n = (MU * k_elem + (1.0 - MU) * cnt_new).astype(np.float32)
    usage = (k_elem_n >= THRESHOLD).astype(np.float32)[:, None]
    k_new = (usage * (k_sum_n / k_elem_n[:, None]) + (1.0 - usage) * k).astype(
        np.float32
    )

    return (xd, codes, commit, k_new, k_sum_n, k_elem_n)


def run_on_hw(x, k, k_sum, k_elem, mode=MATMUL_MODE,
              gpsimd_period=GPSIMD_MAX_PERIOD, trace=False):
    nc = _get_nc(mode, gpsimd_period)
    in_maps = make_in_maps(x, k, mode)
    res = bass_utils.run_bass_kernel_spmd(
        nc, in_maps, core_ids=list(range(NCORES)), trace=trace
    )
    outs = postprocess(res.results, x, k, k_sum, k_elem)
    return outs, res


def kernel(x, k, k_sum, k_elem):
    outs, _ = run_on_hw(x, k, k_sum, k_elem)
    return outs


# revision 38
# speedup vs baseline: 1.6256x; 1.6256x over previous
"""VQ codebook bottleneck block on 8 Trainium2 NeuronCores.

Full inputs in, full outputs out. Data-parallel over the batch axis:
core c handles x[c] (8192 tokens of dim 64); the [2048, 64] codebook is
replicated. The device computes, per core, the argmin codes (via a
single fused matmul S = 2*x@k.T - ||k||^2 and a max/max-index search)
and gathers xq = k[codes]. The tiny EMA/scatter update is combined on
the host.

Self-contained: shapes/sharding hardcoded for
x:[8,64,8192] k:[2048,64] k_sum:[2048,64] k_elem:[2048].
"""

import numpy as np

import concourse.bass as bass
import concourse.bacc as bacc
import concourse.mybir as mybir
from concourse.tile import TileContext
from concourse import bass_utils

MU = 0.99
THRESHOLD = 1.0

N, D, T, K = 8, 64, 8192, 2048
M = T              # tokens per core (one batch entry per core)
MT = M // 128      # m-tiles per core
NCORES = 8
NG = 8             # gather groups
SG = MT // NG      # m-tiles per group

F32 = mybir.dt.float32
F32R = mybir.dt.float32r
BF16 = mybir.dt.bfloat16
FP16 = mybir.dt.float16
U16 = mybir.dt.uint16
I16 = mybir.dt.int16

# matmul mode: "bf16x2" | "fp16x2" (3-term hi/lo splits, 1 cyc/row each
# term), "fp32" (exact, 4 cyc/row), "fp32r" (tf32-class, too lossy).
MATMUL_MODE = "bf16x2"
# m-tiles with s % GPSIMD_MAX_PERIOD == 0 compute the row max via DVE
# max8; others use a GPSIMD tensor_tensor max tree (load balancing).
# 0 => all tiles use DVE max8.
GPSIMD_MAX_PERIOD = 0


def _hi_lo(a, dt):
    np_dt = np.dtype(mybir.dt.np(dt))
    if np_dt == np.float16:
        hi = np.asarray(a, np.float32).astype(np.float16)
        lo = (np.asarray(a, np.float32) - hi.astype(np.float32)).astype(np.float16)
        return hi, lo
    # bf16 via bit truncation with round-to-nearest
    a32 = np.asarray(a, np.float32)
    b = a32.view(np.uint32)
    hi = ((b + 0x8000) & 0xFFFF0000).view(np.float32)
    lo = a32 - hi
    return hi.astype(np_dt), lo.astype(np_dt)


def build_nc(mode=MATMUL_MODE, gpsimd_period=GPSIMD_MAX_PERIOD):
    nc = bacc.Bacc("TRN2", target_bir_lowering=False, debug=False)
    lowp = mode in ("bf16x2", "fp16x2")
    HDT = BF16 if mode == "bf16x2" else FP16

    # ---- I/O ----
    if lowp:
        # packed 2-matmul hi/lo scheme, both at full 128-row contraction:
        #   S = xpa.T @ wa + xpb.T @ wb
        # xpa = [x_hi; x_hi], wa = [w_hi; w_lo]
        # xpb = [x_lo; 1; 1; 0...], wb = [w_hi; -kk_hi; -kk_lo; 0...]
        xpa = nc.dram_tensor("xpa", [128, M], HDT, kind="ExternalInput")
        xpb = nc.dram_tensor("xpb", [128, M], HDT, kind="ExternalInput")
        wa = nc.dram_tensor("wa", [128, K], HDT, kind="ExternalInput")
        wb = nc.dram_tensor("wb", [128, K], HDT, kind="ExternalInput")
    else:
        _xt = F32R if mode == "fp32r" else F32
        xplus = nc.dram_tensor("xplus", [D + 1, M], _xt, kind="ExternalInput")
        w = nc.dram_tensor("w", [D + 1, K], _xt, kind="ExternalInput")
    kg = nc.dram_tensor("kg", [K, D], F32, kind="ExternalInput")

    xq_out = nc.dram_tensor("xq_out", [M, D], F32, kind="ExternalOutput")
    codes_out = nc.dram_tensor("codes_out", [M], U16, kind="ExternalOutput")

    with TileContext(nc) as tc:
        with (
            tc.tile_pool(name="persist", bufs=1) as persist,
            tc.tile_pool(name="spsum", bufs=2, space="PSUM") as spsum,
            tc.tile_pool(name="ssb", bufs=6) as ssb,
            tc.tile_pool(name="small", bufs=8) as small,
            tc.tile_pool(name="gpool", bufs=3) as gpool,
        ):
            # ---- persistent SBUF ----
            MG = M // NG
            xpa_g, xpb_g = [], []
            if lowp:
                wa_sb = persist.tile([128, K], HDT, tag="wa")
                wb_sb = persist.tile([128, K], HDT, tag="wb")
                nc.sync.dma_start(wa_sb[:], wa[:])
                nc.sync.dma_start(wb_sb[:], wb[:])
                for g in range(NG):
                    m_sl = slice(g * MG, (g + 1) * MG)
                    ta = persist.tile([128, MG], HDT, tag=f"xpa{g}")
                    tb = persist.tile([128, MG], HDT, tag=f"xpb{g}")
                    nc.sync.dma_start(ta[:], xpa[:, m_sl])
                    nc.sync.dma_start(tb[:], xpb[:, m_sl])
                    xpa_g.append(ta)
                    xpb_g.append(tb)
            else:
                _xt = F32R if mode == "fp32r" else F32
                w_sb = persist.tile([D + 1, K], _xt, tag="w")
                nc.sync.dma_start(w_sb[:], w[:])
                for g in range(NG):
                    m_sl = slice(g * MG, (g + 1) * MG)
                    tp = persist.tile([D + 1, MG], _xt, tag=f"xp{g}")
                    nc.sync.dma_start(tp[:], xplus[:, m_sl])
                    xpa_g.append(tp)

            codes8_sb = persist.tile([128, MT, 8], U16, tag="codes8")

            # ---- software-pipelined m-tile loop (stage-skewed emission) ----
            # stage A (s): matmuls into PSUM
            # stage B (s-1): ACT copy PSUM -> SBUF
            # stage C (s-2): DVE max8 + max_index
            s_sb_ring = {}
            psum_ring = {}

            def stage_a(s):
                g, sl = divmod(s, SG)
                xsl = slice(sl * 128, (sl + 1) * 128)
                s_ps = spsum.tile([128, K], F32, tag="S")
                for j in range(K // 512):
                    js = slice(j * 512, (j + 1) * 512)
                    ps = s_ps[:, js]
                    if lowp:
                        nc.tensor.matmul(
                            ps, xpa_g[g][:, xsl], wa_sb[:, js],
                            start=True, stop=False,
                        )
                        nc.tensor.matmul(
                            ps, xpb_g[g][:, xsl], wb_sb[:, js],
                            start=False, stop=True,
                        )
                    else:
                        nc.tensor.matmul(
                            ps, xpa_g[g][:, xsl], w_sb[:, js],
                            start=True, stop=True,
                        )
                psum_ring[s] = s_ps

            def stage_b(s):
                s_sb = ssb.tile([128, K], F32, tag="Ssb")
                nc.scalar.copy(s_sb[:], psum_ring.pop(s)[:])
                s_sb_ring[s] = s_sb

            max8_ring = {}

            def stage_c1(s):
                # max8 only; find is skewed one tile behind so the DVE never
                # reads a value its pipe just produced (RAW flush avoidance)
                s_sb = s_sb_ring[s]
                in_max8 = small.tile([128, 8], F32, tag="max8")
                nc.vector.max(in_max8[:], s_sb[:])
                max8_ring[s] = in_max8

            def stage_c2(s):
                s_sb = s_sb_ring.pop(s)
                in_max8 = max8_ring.pop(s)
                nc.vector.max_index(codes8_sb[:, s, :], in_max8[:], s_sb[:])

            def group_tail(g):
                gsl = slice(g * SG, (g + 1) * SG)
                csl = slice(g * (M // NG), (g + 1) * (M // NG))
                idxs_g = gpool.tile([128, M // NG // 16], U16, tag="idxs")
                xq_g = gpool.tile([128, SG, D], F32, tag="xqg")
                # bounce through DRAM to cross partitions (also the output)
                nc.sync.dma_start(
                    codes_out[csl].rearrange("(s p) -> p s", p=128),
                    codes8_sb[:, gsl, 0],
                )
                for r in range(8):
                    nc.sync.dma_start(
                        idxs_g[r * 16:(r + 1) * 16, :],
                        codes_out[csl].rearrange("(c q) -> q c", q=16),
                    )
                nc.gpsimd.dma_gather(
                    xq_g[:], kg[:], idxs_g[:].bitcast(I16),
                    M // NG, M // NG, D, queue_num=0,
                )
                # on the gpsimd queue: naturally ordered after the gather and
                # keeps the sync queue free of head-of-line gather waits
                nc.gpsimd.dma_start(
                    xq_out[:].rearrange("(s p) d -> p s d", p=128)[:, gsl, :],
                    xq_g[:],
                )

            SKEW_B, SKEW_C1, SKEW_C2 = 1, 2, 3
            for s in range(MT + SKEW_C2 + 1):
                if s < MT:
                    stage_a(s)
                if SKEW_B <= s < MT + SKEW_B:
                    stage_b(s - SKEW_B)
                if SKEW_C1 <= s < MT + SKEW_C1:
                    stage_c1(s - SKEW_C1)
                if SKEW_C2 <= s < MT + SKEW_C2:
                    sc = s - SKEW_C2
                    stage_c2(sc)
                    if (sc + 1) % SG == 0:
                        group_tail(sc // SG)

    nc.compile()
    return nc


_NC_CACHE = {}


def _get_nc(mode=MATMUL_MODE, gpsimd_period=GPSIMD_MAX_PERIOD):
    key = (mode, gpsimd_period)
    if key not in _NC_CACHE:
        _NC_CACHE[key] = build_nc(mode, gpsimd_period)
    return _NC_CACHE[key]


def make_in_maps(x, k, mode=MATMUL_MODE):
    x = np.asarray(x, np.float32)
    k = np.asarray(k, np.float32)
    kk = (k.astype(np.float64) ** 2).sum(1).astype(np.float32)
    lowp = mode in ("bf16x2", "fp16x2")
    HDT = BF16 if mode == "bf16x2" else FP16
    np_h = np.dtype(mybir.dt.np(HDT))
    kc = np.ascontiguousarray(k)
    in_maps = []
    if lowp:
        w64 = 2.0 * k.T  # [64, K]
        wh, wl = _hi_lo(w64, HDT)
        kkh, kkl = _hi_lo(-kk[None, :], HDT)
        wa_full = np.concatenate([wh, wl], axis=0)  # [128, K]
        wb_full = np.concatenate(
            [wh, kkh, kkl, np.zeros((62, K), np_h)], axis=0
        )  # [128, K]
        ones = np.ones((1, M), np_h)
        zpad = np.zeros((62, M), np_h)
        for c in range(NCORES):
            xh, xl = _hi_lo(x[c], HDT)
            m = {
                "kg": kc,
                "xpa": np.ascontiguousarray(np.concatenate([xh, xh], axis=0)),
                "xpb": np.ascontiguousarray(
                    np.concatenate([xl, ones, ones, zpad], axis=0)
                ),
                "wa": np.ascontiguousarray(wa_full),
                "wb": np.ascontiguousarray(wb_full),
            }
            in_maps.append(m)
    else:
        w_full = np.concatenate([2.0 * k.T, -kk[None, :]], axis=0)  # [65, K]
        ones = np.ones((1, M), np.float32)
        for c in range(NCORES):
            xplus = np.concatenate([x[c], ones], axis=0)  # [65, M]
            m = {
                "kg": kc,
                "xplus": np.ascontiguousarray(xplus),
                "w": np.ascontiguousarray(w_full),
            }
            in_maps.append(m)
    return in_maps


def postprocess(results, x, k, k_sum, k_elem):
    """results: list of 8 dicts with xq_out, codes_out."""
    x = np.asarray(x, np.float32)
    k = np.asarray(k, np.float32)
    k_sum = np.asarray(k_sum, np.float32)
    k_elem = np.asarray(k_elem, np.float32)

    xq = np.stack([results[c]["xq_out"] for c in range(NCORES)])  # [8, T, D]
    codes = np.stack(
        [results[c]["codes_out"].astype(np.int32) for c in range(NCORES)]
    )  # [8, T]

    xd = np.ascontiguousarray(xq.transpose(0, 2, 1)).astype(np.float32)

    # commitment loss
    xf = x.transpose(0, 2, 1).reshape(-1, D)
    xqf = xq.reshape(-1, D)
    diff = xqf.astype(np.float64) - xf.astype(np.float64)
    commit = np.float32(np.mean(diff * diff))

    # segment sums on host (device scatter-add RMW races on dup indices)
    flat_codes = codes.reshape(-1).astype(np.int64)
    cnt_new = np.bincount(flat_codes, minlength=K).astype(np.float32)
    sum_new = np.empty((K, D), np.float32)
    xf64 = xf.astype(np.float64)
    for d in range(D):
        sum_new[:, d] = np.bincount(flat_codes, weights=xf64[:, d], minlength=K)

    k_sum_n = (MU * k_sum + (1.0 - MU) * sum_new).astype(np.float32)
    k_elem_n = (MU * k_elem + (1.0 - MU) * cnt_new).astype(np.float32)
    usage = (k_elem_n >= THRESHOLD).astype(np.float32)[:, None]
    k_new = (usage * (k_sum_n / k_elem_n[:, None]) + (1.0 - usage) * k).astype(
        np.float32
    )

    return (xd, codes, commit, k_new, k_sum_n, k_elem_n)


def run_on_hw(x, k, k_sum, k_elem, mode=MATMUL_MODE,
              gpsimd_period=GPSIMD_MAX_PERIOD, trace=False):
    nc = _get_nc(mode, gpsimd_period)
    in_maps = make_in_maps(x, k, mode)
    res = bass_utils.run_bass_kernel_spmd(
        nc, in_maps, core_ids=list(range(NCORES)), trace=trace
    )
    outs = postprocess(res.results, x, k, k_sum, k_elem)
    return outs, res


def kernel(x, k, k_sum, k_elem):
    outs, _ = run_on_hw(x, k, k_sum, k_elem)
    return outs
